# revision 15
# baseline (speedup 1.0000x reference)
"""Content-based addressing read (DNC-style) for Trainium2.

Computes softmax_n( strengths[r] * cos_sim(memory[b,n,:], read_vectors[b,:,r]) )
for B=16, N=32768, W=128, R=8, sharded batch-parallel across 8 NeuronCores
(2 batches per core).

v9: elementwise-engine diet on top of v7's 16-bit datapath.
  - norms: squares split GpSimd (scalar_tensor_tensor, 0.6 eff) + ACT
    (off DVE); DVE fold/fold/f32-reduce (measured: TensorReduce has no f16
    2x mode, so the single-reduce variant is slower).
  - 1/sqrt via ACT Sqrt + DVE reciprocal_approx_fast: Ln/Exp thrashed the
    ACT table loads (ln and exp live in different first-match act tables,
    1.28us per swap); Sqrt/Square/Copy share one table.
  - scores stored [128, R, T]: the sim-PSUM drain is a fused DVE
    tensor_tensor multiply by inv_nrm (replaces the ACT score-copy AND the
    softmax-tail normalize pass); s1 reduce becomes contiguous.
  - output written f16 scaled by 2^15 (s1 pre-scaled 2^-15 so totals stay
    in f16 normal range); host casts back to f32 and divides. Halves the
    output-DMA tail; out halves issued on sync+scalar HWDGE queues.
  - memT PSUM drains rotate v,s,s,g across DVE/ACT/GpSimd.
  - gpsimd casting DMA (f32 HBM -> f16 SBUF) with 4-group issue lookahead
    unchanged from v7.
Softmax math stays fp32; no max subtraction (|scores| <= ~1); the reference's
+1e-8 is a provable fp32 no-op (normalizer ~128).

Output is stored in DRAM as (b, p, r, tau) f16*2^15 with n = g*4096 + p*32 + t,
tau = g*32 + t; the host rescales and re-transposes to (b, n, r) f32.
"""

import sys

for _p in ("/opt/trn_rl_repo",):
    if _p not in sys.path:
        sys.path.insert(0, _p)

from contextlib import ExitStack

import numpy as np

import concourse.bass as bass
import concourse.bacc as bacc
import concourse.tile as tile
from concourse import mybir
from concourse import bass_isa
from concourse.bass_utils import run_bass_kernel_spmd

F32 = mybir.dt.float32
F16 = mybir.dt.float16
AF = mybir.ActivationFunctionType
MUL = mybir.AluOpType.mult

B, N, W, R = 16, 32768, 128, 8
NCORES = 8
BLOC = B // NCORES          # batches per core
T = N // 128                # 256 n-tiles of 128 per batch
NG = 8                      # DMA groups per batch
TPG = T // NG               # 32 tiles per group (4096 n, 2MB)
CH = 8                      # tiles per PSUM transpose chunk (1024 cols)
NCH = TPG // CH             # chunks per group
NSTEP = BLOC * NG           # 16 flat steps

# ---- tuning knobs ----
GP_SQ_T = 16                # t-slices squared on GpSimd (rest on ACT)
GP_SQ_SPLIT = 2             # gpsimd square sub-slices (DMA-issue interleave)
MEMT_DRAIN = "vs"           # rotation for memT PSUM->SBUF drains (no PSUM on gp!)
NORM_PATH = "fold"          # "fold" = fd1+fd2+f32 reduce; "dve16" = f16 reduce
DMA_AHEAD = 4               # DMA issue lookahead (must be < IN_BUFS - 2)
IN_BUFS = 9
OUT_SCALE = 32768.0         # output written f16 * 2^15; host divides


def build_program():
    nc = bacc.Bacc("TRN2", target_bir_lowering=False, debug=False, num_devices=NCORES)

    mem = nc.dram_tensor("memory", [BLOC, N, W], F32, kind="ExternalInput").ap()
    rv = nc.dram_tensor("read_vectors", [BLOC, W, R], F32, kind="ExternalInput").ap()
    rs = nc.dram_tensor("read_strengths", [BLOC, R], F32, kind="ExternalInput").ap()
    ident = nc.dram_tensor("identity", [128, 128], F32, kind="ExternalInput").ap()
    ones = nc.dram_tensor("ones", [128, 128], F32, kind="ExternalInput").ap()
    out = nc.dram_tensor("out", [BLOC, 128, R, T], F16, kind="ExternalOutput").ap()

    with ExitStack() as ctx:
        tc = ctx.enter_context(tile.TileContext(nc))

        const_pool = ctx.enter_context(tc.tile_pool(name="const", bufs=1))
        id_t = const_pool.tile([128, 128], F32)
        nc.sync.dma_start(id_t[:], ident)
        ones_t = const_pool.tile([128, 128], F32)
        nc.sync.dma_start(ones_t[:], ones)
        id_h = const_pool.tile([128, 128], F16)
        nc.scalar.copy(id_h[:], id_t[:])
        ones_h = const_pool.tile([128, 128], F16)
        nc.scalar.copy(ones_h[:], ones_t[:])

        in_pool = ctx.enter_context(tc.tile_pool(name="mem_in", bufs=IN_BUFS))
        sq_pool = ctx.enter_context(tc.tile_pool(name="sq", bufs=3))
        fd_pool = ctx.enter_context(tc.tile_pool(name="fd", bufs=2))
        fd2_pool = ctx.enter_context(tc.tile_pool(name="fd2", bufs=2))
        mtps_pool = ctx.enter_context(tc.tile_pool(name="mtps", bufs=4, space="PSUM"))
        mt_pool = ctx.enter_context(tc.tile_pool(name="mt", bufs=6))
        scps_pool = ctx.enter_context(tc.tile_pool(name="scps", bufs=3, space="PSUM"))
        rtps_pool = ctx.enter_context(tc.tile_pool(name="rtps", bufs=1, space="PSUM"))
        smalls = ctx.enter_context(tc.tile_pool(name="smalls", bufs=2))
        score_pool = ctx.enter_context(tc.tile_pool(name="scores", bufs=2))
        scout_pool = ctx.enter_context(tc.tile_pool(name="scout", bufs=2))
        ss_pool = ctx.enter_context(tc.tile_pool(name="ss", bufs=2))
        inv_pool = ctx.enter_context(tc.tile_pool(name="inv", bufs=2))

        state = {"drain_i": 0}

        # per-batch state
        scores_t = [None] * BLOC
        ss_t = [None] * BLOC
        inv_t = [None] * BLOC
        rvp_t = [None] * BLOC
        mem_tiles = {}  # flat step -> mem_g tile
        sq_tiles = {}   # flat step -> sq tile
        scps_tiles = {}  # flat step -> sim psum tile

        def issue_dma(step):
            b, g = divmod(step, NG)
            mem_g = in_pool.tile([128, TPG, W], F16)
            src = mem[b, g * TPG * 128 : (g + 1) * TPG * 128, :].rearrange(
                "(p t) w -> p t w", p=128
            )
            nc.gpsimd.dma_start(mem_g[:], src)  # casting DMA f32->f16
            mem_tiles[step] = mem_g

        def rv_prep(b):
            rv_t = smalls.tile([128, R], F32)
            nc.sync.dma_start(rv_t[:], rv[b])
            rs_t = smalls.tile([1, R], F32)
            nc.sync.dma_start(rs_t[:], rs[b : b + 1, :])
            rs_h = smalls.tile([1, R], F16)
            nc.scalar.copy(rs_h[:], rs_t[:])

            rv2 = smalls.tile([128, R], F16)
            nc.vector.tensor_mul(rv2[:], rv_t[:], rv_t[:])
            nv2_ps = rtps_pool.tile([128, R], F32, tag="prep")
            nc.tensor.matmul(nv2_ps[:], ones_h[:], rv2[:], start=True, stop=True)
            nv = smalls.tile([128, R], F32)
            nc.scalar.activation(nv[:], nv2_ps[:], AF.Sqrt)
            inv_nv = smalls.tile([128, R], F32)
            nc.vector.reciprocal_approx_fast(inv_nv[:], nv[:])
            rsb_ps = rtps_pool.tile([128, R], F32, tag="prep")
            nc.tensor.matmul(
                rsb_ps[:], ones_h[0:1, :], rs_h[:], start=True, stop=True
            )
            factor = smalls.tile([128, R], F32)
            nc.vector.tensor_mul(factor[:], rsb_ps[:], inv_nv[:])
            rvp = smalls.tile([128, R], F32, tag="rvp")
            nc.vector.tensor_mul(rvp[:], rv_t[:], factor[:])
            rvp_h = smalls.tile([128, R], F16, tag="rvph")
            nc.scalar.copy(rvp_h[:], rvp[:])
            rvp_t[b] = rvp_h

        def emit_squares(step):
            """Square mem_g into sq_g, split GpSimd [0:GP_SQ_T] / ACT [GP_SQ_T:]."""
            mem_g = mem_tiles[step]
            sq_g = sq_pool.tile([128, TPG, W], F16)
            sp = GP_SQ_T // GP_SQ_SPLIT
            for k in range(GP_SQ_SPLIT):
                sl = (slice(None), slice(k * sp, (k + 1) * sp), slice(None))
                nc.gpsimd.tensor_mul(sq_g[sl], mem_g[sl], mem_g[sl])
            if GP_SQ_T < TPG:
                sl = (slice(None), slice(GP_SQ_T, TPG), slice(None))
                nc.scalar.square(sq_g[sl], mem_g[sl])
            sq_tiles[step] = sq_g

        def emit_norm(step):
            """Fold w 128->64->32 (DVE f16 2x) then f32 reduce."""
            b, g = divmod(step, NG)
            sq_g = sq_tiles.pop(step)
            ss = ss_t[b]
            if NORM_PATH == "dve16":
                with nc.allow_low_precision("norm sums f16; tol budget 2e-2"):
                    nc.vector.reduce_sum(
                        ss[:, g * TPG : (g + 1) * TPG],
                        sq_g[:],
                        axis=mybir.AxisListType.X,
                    )
            else:
                fd_g = fd_pool.tile([128, TPG, W // 2], F16)
                nc.vector.tensor_add(
                    fd_g[:], sq_g[:, :, 0 : W // 2], sq_g[:, :, W // 2 : W]
                )
                fd2_g = fd2_pool.tile([128, TPG, W // 4], F16)
                nc.vector.tensor_add(
                    fd2_g[:], fd_g[:, :, 0 : W // 4], fd_g[:, :, W // 4 : W // 2]
                )
                nc.vector.reduce_sum(
                    ss[:, g * TPG : (g + 1) * TPG],
                    fd2_g[:],
                    axis=mybir.AxisListType.X,
                )

        def emit_inv(step_lo, step_hi):
            """inv_nrm = 1/sqrt(ss) for groups [step_lo, step_hi)'s tau range.
            ACT Sqrt + DVE fast reciprocal (avoids Ln/Exp act-table thrash)."""
            b, g_lo = divmod(step_lo, NG)
            g_hi = g_lo + (step_hi - step_lo)
            ts = slice(g_lo * TPG, g_hi * TPG)
            ss = ss_t[b]
            inv = inv_t[b]
            nrm = smalls.tile([128, (g_hi - g_lo) * TPG], F32, tag="nrmt")
            nc.scalar.activation(nrm[:], ss[:, ts], AF.Sqrt)
            nc.vector.reciprocal_approx_fast(inv[:, ts], nrm[:])

        def emit_score_drain(step):
            """scores[:, :, tau] = scps * inv_nrm  (fused PSUM drain, DVE)."""
            b, g = divmod(step, NG)
            scps = scps_tiles.pop(step)
            scores = scores_t[b]
            ts = slice(g * TPG, (g + 1) * TPG)
            scps_v = scps[:].rearrange("p (t r) -> p t r", r=R).transpose([0, 2, 1])
            inv_b = (
                inv_t[b][:, ts].unsqueeze(1).broadcast_to([128, R, TPG])
            )
            nc.vector.tensor_mul(scores[:, :, ts], scps_v, inv_b)

        def softmax_tail(b):
            scores = scores_t[b]          # [128, R, T] f32, pre-exp
            # exp per r-slice with ACT accumulator: s1[p, r] = sum_t exp for free
            s1 = smalls.tile([128, R], F32, tag="s1")
            for r in range(R):
                nc.scalar.activation(
                    scores[:, r, :], scores[:, r, :], AF.Exp,
                    accum_out=s1[:, r : r + 1],
                )
            s1_h = smalls.tile([128, R], F16)
            # pre-scale by 2^-15 so f16 output (x OUT_SCALE) stays normal-range
            nc.scalar.activation(s1_h[:], s1[:], AF.Copy, scale=1.0 / OUT_SCALE)
            tot_ps = rtps_pool.tile([128, R], F32, tag="prep")
            nc.tensor.matmul(tot_ps[:], ones_h[:], s1_h[:], start=True, stop=True)
            inv_tot = smalls.tile([128, R], F32, tag="invtot")
            nc.vector.reciprocal_approx_fast(inv_tot[:], tot_ps[:])

            scout = scout_pool.tile([128, R, T], F16, tag="scout")
            H = R // 2
            for h in range(2):
                hs = slice(h * H, (h + 1) * H)
                inv_b = inv_tot[:, hs].unsqueeze(2).broadcast_to([128, H, T])
                nc.vector.tensor_mul(scout[:, hs, :], scores[:, hs, :], inv_b)
                if h == 0:
                    nc.sync.dma_start(out[b, :, hs, :], scout[:, hs, :])
                else:
                    nc.scalar.dma_start(out[b, :, hs, :], scout[:, hs, :])

        def emit_group_mm(step):
            """transposes + memT drains + sim matmuls for one group."""
            b, g = divmod(step, NG)
            mem_g = mem_tiles.pop(step)
            scps = scps_pool.tile([128, TPG * R], F32)
            for q in range(NCH):  # 8-tile chunks (1024 n)
                mt_ps = mtps_pool.tile([128, CH * 128], F16)
                for j in range(CH):
                    tt = q * CH + j
                    nc.tensor.transpose(
                        mt_ps[:, j * 128 : (j + 1) * 128],
                        mem_g[:, tt, :],
                        id_h[:],
                    )
                mt_sb = mt_pool.tile([128, CH * 128], F16)
                de = MEMT_DRAIN[state["drain_i"] % len(MEMT_DRAIN)]
                state["drain_i"] += 1
                if de == "s":
                    nc.scalar.copy(mt_sb[:], mt_ps[:])
                else:
                    nc.vector.tensor_copy(mt_sb[:], mt_ps[:])

                for j in range(CH):
                    tt = q * CH + j
                    nc.tensor.matmul(
                        scps[:, tt * R : (tt + 1) * R],
                        mt_sb[:, j * 128 : (j + 1) * 128],
                        rvp_t[b][:],
                        start=True,
                        stop=True,
                    )
            scps_tiles[step] = scps

        # prologue: fill DMA pipeline, prep batch 0
        for s0 in range(DMA_AHEAD):
            issue_dma(s0)
        rv_prep(0)

        for step in range(NSTEP):
            b, g = divmod(step, NG)
            if g == 0:
                scores_new = score_pool.tile([128, R, T], F32, tag="scores")
                ss_new = ss_pool.tile([128, T], F32, tag="ss")
                inv_new = inv_pool.tile([128, T], F32, tag="inv")
                scores_t[b] = scores_new
                ss_t[b] = ss_new
                inv_t[b] = inv_new
            if step + DMA_AHEAD < NSTEP:
                issue_dma(step + DMA_AHEAD)
            # batch b+1 rv-prep midway through batch b
            if g == NG - 3 and b + 1 < BLOC:
                rv_prep(b + 1)

            emit_squares(step)
            # deferred norm reduce (1 step) keeps DVE off the DMA chase
            if step >= 1:
                emit_norm(step - 1)
            # inv_nrm for the previous two groups, every other step
            if step >= 2 and step % 2 == 0:
                emit_inv(step - 2, step)

            emit_group_mm(step)

            # fused score drain two steps back (inv_nrm ready by then)
            if step >= 2:
                emit_score_drain(step - 2)
            # batch b-1 softmax tail overlapped into batch b's third group
            if g == 2 and b > 0:
                softmax_tail(b - 1)

        # epilogue: flush remaining norms, invs, drains + last batch softmax
        emit_norm(NSTEP - 1)
        emit_inv(NSTEP - 2, NSTEP)
        emit_score_drain(NSTEP - 2)
        emit_score_drain(NSTEP - 1)
        softmax_tail(BLOC - 1)

    nc.compile()
    return nc


_program = None
last_results = None


def _get_program():
    global _program
    if _program is None:
        _program = build_program()
    return _program


def kernel(memory, read_strengths, read_vectors):
    memory = np.asarray(memory, dtype=np.float32)
    read_strengths = np.asarray(read_strengths, dtype=np.float32)
    read_vectors = np.asarray(read_vectors, dtype=np.float32)

    nc = _get_program()
    identity = np.eye(128, dtype=np.float32)
    ones_m = np.ones((128, 128), dtype=np.float32)
    in_maps = []
    for c in range(NCORES):
        sl = slice(c * BLOC, (c + 1) * BLOC)
        in_maps.append(
            {
                "memory": np.ascontiguousarray(memory[sl]),
                "read_vectors": np.ascontiguousarray(read_vectors[sl]),
                "read_strengths": np.ascontiguousarray(read_strengths[sl]),
                "identity": identity,
                "ones": ones_m,
            }
        )

    global last_results
    last_results = run_bass_kernel_spmd(nc, in_maps, list(range(NCORES)))
    res = last_results.results
    outs = []
    for c in range(NCORES):
        # (BLOC, 128, R, T) f16 * 2^15; tau = g*TPG + t; n = g*4096 + p*32 + t
        o = np.asarray(res[c]["out"]).astype(np.float32) / OUT_SCALE
        o = o.reshape(BLOC, 128, R, NG, TPG).transpose(0, 3, 1, 4, 2)
        outs.append(o.reshape(BLOC, N, R))
    return np.concatenate(outs, axis=0)


# revision 22
# speedup vs baseline: 1.0402x; 1.0402x over previous
"""Content-based addressing read (DNC-style) for Trainium2.

Computes softmax_n( strengths[r] * cos_sim(memory[b,n,:], read_vectors[b,:,r]) )
for B=16, N=32768, W=128, R=8, sharded batch-parallel across 8 NeuronCores
(2 batches per core).

v9: elementwise-engine diet on top of v7's 16-bit datapath.
  - norms: squares split GpSimd (scalar_tensor_tensor, 0.6 eff) + ACT
    (off DVE); DVE fold/fold/f32-reduce (measured: TensorReduce has no f16
    2x mode, so the single-reduce variant is slower).
  - 1/sqrt via ACT Sqrt + DVE reciprocal_approx_fast: Ln/Exp thrashed the
    ACT table loads (ln and exp live in different first-match act tables,
    1.28us per swap); Sqrt/Square/Copy share one table.
  - scores stored [128, R, T]: the sim-PSUM drain is a fused DVE
    tensor_tensor multiply by inv_nrm (replaces the ACT score-copy AND the
    softmax-tail normalize pass); s1 reduce becomes contiguous.
  - output written f16 scaled by 2^15 (s1 pre-scaled 2^-15 so totals stay
    in f16 normal range); host casts back to f32 and divides. Halves the
    output-DMA tail; out halves issued on sync+scalar HWDGE queues.
  - memT PSUM drains rotate v,s,s,g across DVE/ACT/GpSimd.
  - gpsimd casting DMA (f32 HBM -> f16 SBUF) with 4-group issue lookahead
    unchanged from v7.
Softmax math stays fp32; no max subtraction (|scores| <= ~1); the reference's
+1e-8 is a provable fp32 no-op (normalizer ~128).

Output is stored in DRAM as (b, p, r, tau) f16*2^15 with n = g*4096 + p*32 + t,
tau = g*32 + t; the host rescales and re-transposes to (b, n, r) f32.
"""

import sys

for _p in ("/opt/trn_rl_repo",):
    if _p not in sys.path:
        sys.path.insert(0, _p)

from contextlib import ExitStack

import numpy as np

import concourse.bass as bass
import concourse.bacc as bacc
import concourse.tile as tile
from concourse import mybir
from concourse import bass_isa
from concourse.bass_utils import run_bass_kernel_spmd

F32 = mybir.dt.float32
F16 = mybir.dt.float16
AF = mybir.ActivationFunctionType
MUL = mybir.AluOpType.mult

B, N, W, R = 16, 32768, 128, 8
NCORES = 8
BLOC = B // NCORES          # batches per core
T = N // 128                # 256 n-tiles of 128 per batch
NG = 8                      # DMA groups per batch
TPG = T // NG               # 32 tiles per group (4096 n, 2MB)
CH = 8                      # tiles per PSUM transpose chunk (1024 cols)
NCH = TPG // CH             # chunks per group
NSTEP = BLOC * NG           # 16 flat steps

# ---- tuning knobs ----
GP_SQ_T = 16                # t-slices squared on GpSimd
DVE_SQ_T = 8                # t-slices squared on DVE, deferred 1 step (2x probe)
GP_SQ_SPLIT = 2             # gpsimd square sub-slices (DMA-issue interleave)
MEMT_DRAIN = "vs"           # rotation for memT PSUM->SBUF drains (no PSUM on gp!)
DMA_AHEAD = 5               # DMA issue lookahead (must be < IN_BUFS - 2)
IN_BUFS = 10
OUT_SCALE = 32768.0         # output written f16 * 2^15; host divides


def build_program():
    nc = bacc.Bacc("TRN2", target_bir_lowering=False, debug=False, num_devices=NCORES)

    mem = nc.dram_tensor("memory", [BLOC, N, W], F32, kind="ExternalInput").ap()
    rv = nc.dram_tensor("read_vectors", [BLOC, W, R], F32, kind="ExternalInput").ap()
    rs = nc.dram_tensor("read_strengths", [BLOC, R], F32, kind="ExternalInput").ap()
    ident = nc.dram_tensor("identity", [128, 128], F32, kind="ExternalInput").ap()
    ones = nc.dram_tensor("ones", [128, 128], F32, kind="ExternalInput").ap()
    out = nc.dram_tensor("out", [BLOC, 128, R, T], F16, kind="ExternalOutput").ap()

    with ExitStack() as ctx:
        tc = ctx.enter_context(tile.TileContext(nc))

        const_pool = ctx.enter_context(tc.tile_pool(name="const", bufs=1))
        id_t = const_pool.tile([128, 128], F32)
        nc.sync.dma_start(id_t[:], ident)
        ones_t = const_pool.tile([128, 128], F32)
        nc.sync.dma_start(ones_t[:], ones)
        id_h = const_pool.tile([128, 128], F16)
        nc.scalar.copy(id_h[:], id_t[:])
        ones_h = const_pool.tile([128, 128], F16)
        nc.scalar.copy(ones_h[:], ones_t[:])

        in_pool = ctx.enter_context(tc.tile_pool(name="mem_in", bufs=IN_BUFS))
        sq_pool = ctx.enter_context(tc.tile_pool(name="sq", bufs=3))
        fd_pool = ctx.enter_context(tc.tile_pool(name="fd", bufs=2))
        fd2_pool = ctx.enter_context(tc.tile_pool(name="fd2", bufs=2))
        fd3_pool = ctx.enter_context(tc.tile_pool(name="fd3", bufs=2))
        mtps_pool = ctx.enter_context(tc.tile_pool(name="mtps", bufs=4, space="PSUM"))
        mt_pool = ctx.enter_context(tc.tile_pool(name="mt", bufs=6))
        scps_pool = ctx.enter_context(tc.tile_pool(name="scps", bufs=3, space="PSUM"))
        rtps_pool = ctx.enter_context(tc.tile_pool(name="rtps", bufs=1, space="PSUM"))
        smalls = ctx.enter_context(tc.tile_pool(name="smalls", bufs=2))
        score_pool = ctx.enter_context(tc.tile_pool(name="scores", bufs=2))
        scout_pool = ctx.enter_context(tc.tile_pool(name="scout", bufs=2))
        ss_pool = ctx.enter_context(tc.tile_pool(name="ss", bufs=2))
        inv_pool = ctx.enter_context(tc.tile_pool(name="inv", bufs=2))

        state = {"drain_i": 0}

        # per-batch state
        scores_t = [None] * BLOC
        ss_t = [None] * BLOC
        inv_t = [None] * BLOC
        rvp_t = [None] * BLOC
        mem_tiles = {}  # flat step -> mem_g tile
        sq_tiles = {}   # flat step -> sq tile
        scps_tiles = {}  # flat step -> sim psum tile

        def issue_dma(step):
            b, g = divmod(step, NG)
            mem_g = in_pool.tile([128, TPG, W], F16)
            src = mem[b, g * TPG * 128 : (g + 1) * TPG * 128, :].rearrange(
                "(p t) w -> p t w", p=128
            )
            nc.gpsimd.dma_start(mem_g[:], src)  # casting DMA f32->f16
            mem_tiles[step] = mem_g

        def rv_prep(b):
            rv_t = smalls.tile([128, R], F32)
            nc.sync.dma_start(rv_t[:], rv[b])
            rs_t = smalls.tile([1, R], F32)
            nc.sync.dma_start(rs_t[:], rs[b : b + 1, :])
            rs_h = smalls.tile([1, R], F16)
            nc.scalar.copy(rs_h[:], rs_t[:])

            rv2 = smalls.tile([128, R], F16)
            nc.vector.tensor_mul(rv2[:], rv_t[:], rv_t[:])
            nv2_ps = rtps_pool.tile([128, R], F32, tag="prep")
            nc.tensor.matmul(nv2_ps[:], ones_h[:], rv2[:], start=True, stop=True)
            nv = smalls.tile([128, R], F32)
            nc.scalar.activation(nv[:], nv2_ps[:], AF.Sqrt)
            inv_nv = smalls.tile([128, R], F32)
            nc.vector.reciprocal_approx_fast(inv_nv[:], nv[:])
            rsb_ps = rtps_pool.tile([128, R], F32, tag="prep")
            nc.tensor.matmul(
                rsb_ps[:], ones_h[0:1, :], rs_h[:], start=True, stop=True
            )
            factor = smalls.tile([128, R], F32)
            nc.vector.tensor_mul(factor[:], rsb_ps[:], inv_nv[:])
            rvp = smalls.tile([128, R], F32, tag="rvp")
            nc.vector.tensor_mul(rvp[:], rv_t[:], factor[:])
            rvp_h = smalls.tile([128, R], F16, tag="rvph")
            nc.scalar.copy(rvp_h[:], rvp[:])
            rvp_t[b] = rvp_h

        def emit_squares(step):
            """Square mem_g: GpSimd [0:GP_SQ_T], ACT [GP_SQ_T:TPG-DVE_SQ_T].
            The DVE slice [TPG-DVE_SQ_T:] is emitted one step later (deferred,
            chase-free) in emit_norm."""
            mem_g = mem_tiles[step]
            sq_g = sq_pool.tile([128, TPG, W], F16)
            sp = GP_SQ_T // GP_SQ_SPLIT
            for k in range(GP_SQ_SPLIT):
                sl = (slice(None), slice(k * sp, (k + 1) * sp), slice(None))
                nc.gpsimd.tensor_mul(sq_g[sl], mem_g[sl], mem_g[sl])
            if GP_SQ_T + DVE_SQ_T < TPG:
                sl = (slice(None), slice(GP_SQ_T, TPG - DVE_SQ_T), slice(None))
                nc.scalar.square(sq_g[sl], mem_g[sl])
            sq_tiles[step] = sq_g

        def emit_norm(step):
            """Deferred DVE square slice, then fold w 128->64->32->16 (f16 2x)
            and a f32 reduce of the last 16."""
            b, g = divmod(step, NG)
            sq_g = sq_tiles.pop(step)
            mem_g = mem_tiles.pop(step)
            ss = ss_t[b]
            if DVE_SQ_T > 0:
                sl = (slice(None), slice(TPG - DVE_SQ_T, TPG), slice(None))
                nc.vector.tensor_mul(sq_g[sl], mem_g[sl], mem_g[sl])
            fd_g = fd_pool.tile([128, TPG, W // 2], F16)
            nc.vector.tensor_add(
                fd_g[:], sq_g[:, :, 0 : W // 2], sq_g[:, :, W // 2 : W]
            )
            fd2_g = fd2_pool.tile([128, TPG, W // 4], F16)
            nc.vector.tensor_add(
                fd2_g[:], fd_g[:, :, 0 : W // 4], fd_g[:, :, W // 4 : W // 2]
            )
            fd3_g = fd3_pool.tile([128, TPG, W // 8], F16)
            nc.vector.tensor_add(
                fd3_g[:], fd2_g[:, :, 0 : W // 8], fd2_g[:, :, W // 8 : W // 4]
            )
            nc.vector.reduce_sum(
                ss[:, g * TPG : (g + 1) * TPG],
                fd3_g[:],
                axis=mybir.AxisListType.X,
            )

        def emit_inv(step_lo, step_hi):
            """inv_nrm = 1/sqrt(ss) for groups [step_lo, step_hi)'s tau range.
            ACT Sqrt + DVE fast reciprocal (avoids Ln/Exp act-table thrash)."""
            b, g_lo = divmod(step_lo, NG)
            g_hi = g_lo + (step_hi - step_lo)
            ts = slice(g_lo * TPG, g_hi * TPG)
            ss = ss_t[b]
            inv = inv_t[b]
            nrm = smalls.tile([128, (g_hi - g_lo) * TPG], F32, tag="nrmt")
            nc.scalar.activation(nrm[:], ss[:, ts], AF.Sqrt)
            nc.vector.reciprocal_approx_fast(inv[:, ts], nrm[:])

        def emit_score_drain(step):
            """scores[:, :, tau] = scps * inv_nrm  (fused PSUM drain, DVE)."""
            b, g = divmod(step, NG)
            scps = scps_tiles.pop(step)
            scores = scores_t[b]
            ts = slice(g * TPG, (g + 1) * TPG)
            scps_v = scps[:].rearrange("p (t r) -> p t r", r=R).transpose([0, 2, 1])
            inv_b = (
                inv_t[b][:, ts].unsqueeze(1).broadcast_to([128, R, TPG])
            )
            nc.vector.tensor_mul(scores[:, :, ts], scps_v, inv_b)

        inv_tot_t = [None] * BLOC

        def softmax_tail_a(b):
            """exp + sum + 1/total (ACT + DVE + PE)."""
            scores = scores_t[b]          # [128, R, T] f32, pre-exp
            nc.scalar.activation(scores[:], scores[:], AF.Exp)
            s1 = smalls.tile([128, R], F32, tag="s1")
            nc.vector.reduce_sum(s1[:], scores[:], axis=mybir.AxisListType.X)
            s1_h = smalls.tile([128, R], F16)
            # pre-scale by 2^-15 so f16 output (x OUT_SCALE) stays normal-range
            nc.scalar.activation(s1_h[:], s1[:], AF.Copy, scale=1.0 / OUT_SCALE)
            tot_ps = rtps_pool.tile([128, R], F32, tag="prep")
            nc.tensor.matmul(tot_ps[:], ones_h[:], s1_h[:], start=True, stop=True)
            inv_tot = smalls.tile([128, R], F32, tag="invtot")
            nc.vector.reciprocal_approx_fast(inv_tot[:], tot_ps[:])
            inv_tot_t[b] = inv_tot

        def softmax_tail_b(b):
            """normalize halves + output DMA (DVE + sync/scalar queues)."""
            scores = scores_t[b]
            inv_tot = inv_tot_t[b]
            scout = scout_pool.tile([128, R, T], F16, tag="scout")
            H = R // 2
            for h in range(2):
                hs = slice(h * H, (h + 1) * H)
                inv_b = inv_tot[:, hs].unsqueeze(2).broadcast_to([128, H, T])
                nc.vector.tensor_mul(scout[:, hs, :], scores[:, hs, :], inv_b)
                if h == 0:
                    nc.sync.dma_start(out[b, :, hs, :], scout[:, hs, :])
                else:
                    nc.scalar.dma_start(out[b, :, hs, :], scout[:, hs, :])

        def emit_group_mm(step):
            """transposes + memT drains + sim matmuls for one group."""
            b, g = divmod(step, NG)
            mem_g = mem_tiles[step]  # popped later by emit_norm (deferred DVE sq)
            scps = scps_pool.tile([128, TPG * R], F32)
            for q in range(NCH):  # 8-tile chunks (1024 n)
                mt_ps = mtps_pool.tile([128, CH * 128], F16)
                for j in range(CH):
                    tt = q * CH + j
                    nc.tensor.transpose(
                        mt_ps[:, j * 128 : (j + 1) * 128],
                        mem_g[:, tt, :],
                        id_h[:],
                    )
                mt_sb = mt_pool.tile([128, CH * 128], F16)
                de = MEMT_DRAIN[state["drain_i"] % len(MEMT_DRAIN)]
                state["drain_i"] += 1
                if de == "s":
                    nc.scalar.copy(mt_sb[:], mt_ps[:])
                else:
                    nc.vector.tensor_copy(mt_sb[:], mt_ps[:])

                for j in range(CH):
                    tt = q * CH + j
                    nc.tensor.matmul(
                        scps[:, tt * R : (tt + 1) * R],
                        mt_sb[:, j * 128 : (j + 1) * 128],
                        rvp_t[b][:],
                        start=True,
                        stop=True,
                    )
            scps_tiles[step] = scps

        # prologue: fill DMA pipeline, prep batch 0
        for s0 in range(DMA_AHEAD):
            issue_dma(s0)
        rv_prep(0)

        for step in range(NSTEP):
            b, g = divmod(step, NG)
            if g == 0:
                scores_new = score_pool.tile([128, R, T], F32, tag="scores")
                ss_new = ss_pool.tile([128, T], F32, tag="ss")
                inv_new = inv_pool.tile([128, T], F32, tag="inv")
                scores_t[b] = scores_new
                ss_t[b] = ss_new
                inv_t[b] = inv_new
            if step + DMA_AHEAD < NSTEP:
                issue_dma(step + DMA_AHEAD)
            # batch b+1 rv-prep midway through batch b
            if g == NG - 3 and b + 1 < BLOC:
                rv_prep(b + 1)

            emit_squares(step)
            # deferred norm chain (1 step) keeps DVE off the DMA chase
            if step >= 1:
                emit_norm(step - 1)
            # inv_nrm for the previous two groups, every other step
            if step >= 2 and step % 2 == 0:
                emit_inv(step - 2, step)
            # fused score drain (ready work) BEFORE this group's drains
            if step >= 2:
                emit_score_drain(step - 2)
            # batch b-1 softmax tail split across groups 2 and 3
            if g == 2 and b > 0:
                softmax_tail_a(b - 1)

            emit_group_mm(step)

            if g == 3 and b > 0:
                softmax_tail_b(b - 1)

        # epilogue: flush remaining norms, invs, drains + last batch softmax
        emit_norm(NSTEP - 1)
        emit_inv(NSTEP - 2, NSTEP)
        emit_score_drain(NSTEP - 2)
        emit_score_drain(NSTEP - 1)
        softmax_tail_a(BLOC - 1)
        softmax_tail_b(BLOC - 1)

    nc.compile()
    return nc


_program = None
last_results = None


def _get_program():
    global _program
    if _program is None:
        _program = build_program()
    return _program


def kernel(memory, read_strengths, read_vectors):
    memory = np.asarray(memory, dtype=np.float32)
    read_strengths = np.asarray(read_strengths, dtype=np.float32)
    read_vectors = np.asarray(read_vectors, dtype=np.float32)

    nc = _get_program()
    identity = np.eye(128, dtype=np.float32)
    ones_m = np.ones((128, 128), dtype=np.float32)
    in_maps = []
    for c in range(NCORES):
        sl = slice(c * BLOC, (c + 1) * BLOC)
        in_maps.append(
            {
                "memory": np.ascontiguousarray(memory[sl]),
                "read_vectors": np.ascontiguousarray(read_vectors[sl]),
                "read_strengths": np.ascontiguousarray(read_strengths[sl]),
                "identity": identity,
                "ones": ones_m,
            }
        )

    global last_results
    last_results = run_bass_kernel_spmd(nc, in_maps, list(range(NCORES)))
    res = last_results.results
    outs = []
    for c in range(NCORES):
        # (BLOC, 128, R, T) f16 * 2^15; tau = g*TPG + t; n = g*4096 + p*32 + t
        o = np.asarray(res[c]["out"]).astype(np.float32) / OUT_SCALE
        o = o.reshape(BLOC, 128, R, NG, TPG).transpose(0, 3, 1, 4, 2)
        outs.append(o.reshape(BLOC, N, R))
    return np.concatenate(outs, axis=0)


# revision 25
# speedup vs baseline: 1.1754x; 1.1300x over previous
"""Content-based addressing read (DNC-style) for Trainium2.

Computes softmax_n( strengths[r] * cos_sim(memory[b,n,:], read_vectors[b,:,r]) )
for B=16, N=32768, W=128, R=8, sharded batch-parallel across 8 NeuronCores
(2 batches per core).

v9: elementwise-engine diet on top of v7's 16-bit datapath.
  - norms: squares split GpSimd (scalar_tensor_tensor, 0.6 eff) + ACT
    (off DVE); DVE fold/fold/f32-reduce (measured: TensorReduce has no f16
    2x mode, so the single-reduce variant is slower).
  - 1/sqrt via ACT Sqrt + DVE reciprocal_approx_fast: Ln/Exp thrashed the
    ACT table loads (ln and exp live in different first-match act tables,
    1.28us per swap); Sqrt/Square/Copy share one table.
  - scores stored [128, R, T]: the sim-PSUM drain is a fused DVE
    tensor_tensor multiply by inv_nrm (replaces the ACT score-copy AND the
    softmax-tail normalize pass); s1 reduce becomes contiguous.
  - output written f16 scaled by 2^15 (s1 pre-scaled 2^-15 so totals stay
    in f16 normal range); host casts back to f32 and divides. Halves the
    output-DMA tail; out halves issued on sync+scalar HWDGE queues.
  - memT PSUM drains rotate v,s,s,g across DVE/ACT/GpSimd.
  - gpsimd casting DMA (f32 HBM -> f16 SBUF) with 4-group issue lookahead
    unchanged from v7.
Softmax math stays fp32; no max subtraction (|scores| <= ~1); the reference's
+1e-8 is a provable fp32 no-op (normalizer ~128).

Output is stored in DRAM as (b, p, r, tau) f16*2^15 with n = g*4096 + p*32 + t,
tau = g*32 + t; the host rescales and re-transposes to (b, n, r) f32.
"""

import sys

for _p in ("/opt/trn_rl_repo",):
    if _p not in sys.path:
        sys.path.insert(0, _p)

from contextlib import ExitStack

import numpy as np

import concourse.bass as bass
import concourse.bacc as bacc
import concourse.tile as tile
from concourse import mybir
from concourse import bass_isa
from concourse.bass_utils import run_bass_kernel_spmd

F32 = mybir.dt.float32
F16 = mybir.dt.float16
AF = mybir.ActivationFunctionType
MUL = mybir.AluOpType.mult

B, N, W, R = 16, 32768, 128, 8
NCORES = 8
BLOC = B // NCORES          # batches per core
T = N // 128                # 256 n-tiles of 128 per batch
NG = 8                      # DMA groups per batch
TPG = T // NG               # 32 tiles per group (4096 n, 2MB)
CH = 8                      # tiles per PSUM transpose chunk (1024 cols)
NCH = TPG // CH             # chunks per group
NSTEP = BLOC * NG           # 16 flat steps

# ---- tuning knobs ----
GP_SQ_T = 16                # t-slices squared on GpSimd
DVE_SQ_T = 0                # t-slices squared on DVE (1x + stalls; keep 0)
GP_SQ_SPLIT = 2             # gpsimd square sub-slices (DMA-issue interleave)
MEMT_DRAIN = "vsvsvsss"     # rotation for memT PSUM->SBUF drains (no PSUM on gp!)
DMA_SPLIT = 2               # casting DMAs per group (earlier consumer wakeup)
DMA_AHEAD = 5               # DMA issue lookahead (must be < IN_BUFS - 2)
IN_BUFS = 10
OUT_SCALE = 32768.0         # output written f16 * 2^15; host divides


def build_program():
    nc = bacc.Bacc("TRN2", target_bir_lowering=False, debug=False, num_devices=NCORES)

    mem = nc.dram_tensor("memory", [BLOC, N, W], F32, kind="ExternalInput").ap()
    rv = nc.dram_tensor("read_vectors", [BLOC, W, R], F32, kind="ExternalInput").ap()
    rs = nc.dram_tensor("read_strengths", [BLOC, R], F32, kind="ExternalInput").ap()
    ident = nc.dram_tensor("identity", [128, 128], F32, kind="ExternalInput").ap()
    ones = nc.dram_tensor("ones", [128, 128], F32, kind="ExternalInput").ap()
    out = nc.dram_tensor("out", [BLOC, 128, R, T], F16, kind="ExternalOutput").ap()

    with ExitStack() as ctx:
        tc = ctx.enter_context(tile.TileContext(nc))

        const_pool = ctx.enter_context(tc.tile_pool(name="const", bufs=1))
        id_t = const_pool.tile([128, 128], F32)
        nc.sync.dma_start(id_t[:], ident)
        ones_t = const_pool.tile([128, 128], F32)
        nc.sync.dma_start(ones_t[:], ones)
        id_h = const_pool.tile([128, 128], F16)
        nc.scalar.copy(id_h[:], id_t[:])
        ones_h = const_pool.tile([128, 128], F16)
        nc.scalar.copy(ones_h[:], ones_t[:])

        in_pool = ctx.enter_context(tc.tile_pool(name="mem_in", bufs=IN_BUFS))
        sq_pool = ctx.enter_context(tc.tile_pool(name="sq", bufs=3))
        fd_pool = ctx.enter_context(tc.tile_pool(name="fd", bufs=2))
        fd2_pool = ctx.enter_context(tc.tile_pool(name="fd2", bufs=2))
        fd3_pool = ctx.enter_context(tc.tile_pool(name="fd3", bufs=2))
        mtps_pool = ctx.enter_context(tc.tile_pool(name="mtps", bufs=4, space="PSUM"))
        mt_pool = ctx.enter_context(tc.tile_pool(name="mt", bufs=6))
        scps_pool = ctx.enter_context(tc.tile_pool(name="scps", bufs=3, space="PSUM"))
        rtps_pool = ctx.enter_context(tc.tile_pool(name="rtps", bufs=1, space="PSUM"))
        smalls = ctx.enter_context(tc.tile_pool(name="smalls", bufs=2))
        score_pool = ctx.enter_context(tc.tile_pool(name="scores", bufs=2))
        scout_pool = ctx.enter_context(tc.tile_pool(name="scout", bufs=2))
        ss_pool = ctx.enter_context(tc.tile_pool(name="ss", bufs=2))
        inv_pool = ctx.enter_context(tc.tile_pool(name="inv", bufs=2))

        state = {"drain_i": 0}

        # per-batch state
        scores_t = [None] * BLOC
        ss_t = [None] * BLOC
        inv_t = [None] * BLOC
        rvp_t = [None] * BLOC
        mem_tiles = {}  # flat step -> mem_g tile
        sq_tiles = {}   # flat step -> sq tile
        scps_tiles = {}  # flat step -> sim psum tile

        def issue_dma(step):
            b, g = divmod(step, NG)
            mem_g = in_pool.tile([128, TPG, W], F16)
            src = mem[b, g * TPG * 128 : (g + 1) * TPG * 128, :].rearrange(
                "(p t) w -> p t w", p=128
            )
            hp = TPG // DMA_SPLIT
            for k in range(DMA_SPLIT):  # split: consumers wake at sub-group grain
                ts = slice(k * hp, (k + 1) * hp)
                nc.gpsimd.dma_start(mem_g[:, ts, :], src[:, ts, :])
            mem_tiles[step] = mem_g

        def rv_prep(b):
            rv_t = smalls.tile([128, R], F32)
            nc.sync.dma_start(rv_t[:], rv[b])
            rs_t = smalls.tile([1, R], F32)
            nc.sync.dma_start(rs_t[:], rs[b : b + 1, :])
            rs_h = smalls.tile([1, R], F16)
            nc.scalar.copy(rs_h[:], rs_t[:])

            rv2 = smalls.tile([128, R], F16)
            nc.vector.tensor_mul(rv2[:], rv_t[:], rv_t[:])
            nv2_ps = rtps_pool.tile([128, R], F32, tag="prep")
            nc.tensor.matmul(nv2_ps[:], ones_h[:], rv2[:], start=True, stop=True)
            nv = smalls.tile([128, R], F32)
            nc.scalar.activation(nv[:], nv2_ps[:], AF.Sqrt)
            inv_nv = smalls.tile([128, R], F32)
            nc.vector.reciprocal_approx_fast(inv_nv[:], nv[:])
            rsb_ps = rtps_pool.tile([128, R], F32, tag="prep")
            nc.tensor.matmul(
                rsb_ps[:], ones_h[0:1, :], rs_h[:], start=True, stop=True
            )
            factor = smalls.tile([128, R], F32)
            nc.vector.tensor_mul(factor[:], rsb_ps[:], inv_nv[:])
            rvp = smalls.tile([128, R], F32, tag="rvp")
            nc.vector.tensor_mul(rvp[:], rv_t[:], factor[:])
            rvp_h = smalls.tile([128, R], F16, tag="rvph")
            nc.scalar.copy(rvp_h[:], rvp[:])
            rvp_t[b] = rvp_h

        def emit_squares(step):
            """Square mem_g: GpSimd [0:GP_SQ_T], ACT [GP_SQ_T:TPG-DVE_SQ_T].
            The DVE slice [TPG-DVE_SQ_T:] is emitted one step later (deferred,
            chase-free) in emit_norm."""
            mem_g = mem_tiles[step]
            sq_g = sq_pool.tile([128, TPG, W], F16)
            sp = GP_SQ_T // GP_SQ_SPLIT
            for k in range(GP_SQ_SPLIT):
                sl = (slice(None), slice(k * sp, (k + 1) * sp), slice(None))
                nc.gpsimd.tensor_mul(sq_g[sl], mem_g[sl], mem_g[sl])
            if GP_SQ_T + DVE_SQ_T < TPG:
                sl = (slice(None), slice(GP_SQ_T, TPG - DVE_SQ_T), slice(None))
                nc.scalar.square(sq_g[sl], mem_g[sl])
            sq_tiles[step] = sq_g

        def emit_norm(step):
            """Deferred DVE square slice, then fold w 128->64->32->16 (f16 2x)
            and a f32 reduce of the last 16."""
            b, g = divmod(step, NG)
            sq_g = sq_tiles.pop(step)
            mem_g = mem_tiles.pop(step)
            ss = ss_t[b]
            if DVE_SQ_T > 0:
                sl = (slice(None), slice(TPG - DVE_SQ_T, TPG), slice(None))
                nc.vector.tensor_mul(sq_g[sl], mem_g[sl], mem_g[sl])
            fd_g = fd_pool.tile([128, TPG, W // 2], F16)
            nc.vector.tensor_add(
                fd_g[:], sq_g[:, :, 0 : W // 2], sq_g[:, :, W // 2 : W]
            )
            fd2_g = fd2_pool.tile([128, TPG, W // 4], F16)
            nc.vector.tensor_add(
                fd2_g[:], fd_g[:, :, 0 : W // 4], fd_g[:, :, W // 4 : W // 2]
            )
            fd3_g = fd3_pool.tile([128, TPG, W // 8], F16)
            nc.vector.tensor_add(
                fd3_g[:], fd2_g[:, :, 0 : W // 8], fd2_g[:, :, W // 8 : W // 4]
            )
            nc.vector.reduce_sum(
                ss[:, g * TPG : (g + 1) * TPG],
                fd3_g[:],
                axis=mybir.AxisListType.X,
            )

        def emit_inv(step_lo, step_hi):
            """inv_nrm = 1/sqrt(ss) for groups [step_lo, step_hi)'s tau range.
            ACT Sqrt + DVE fast reciprocal (avoids Ln/Exp act-table thrash)."""
            b, g_lo = divmod(step_lo, NG)
            g_hi = g_lo + (step_hi - step_lo)
            ts = slice(g_lo * TPG, g_hi * TPG)
            ss = ss_t[b]
            inv = inv_t[b]
            nrm = smalls.tile([128, (g_hi - g_lo) * TPG], F32, tag="nrmt")
            nc.scalar.activation(nrm[:], ss[:, ts], AF.Sqrt)
            nc.vector.reciprocal_approx_fast(inv[:, ts], nrm[:])

        def emit_score_drain(step):
            """scores[:, :, tau] = scps * inv_nrm  (fused PSUM drain, DVE)."""
            b, g = divmod(step, NG)
            scps = scps_tiles.pop(step)
            scores = scores_t[b]
            ts = slice(g * TPG, (g + 1) * TPG)
            scps_v = scps[:].rearrange("p (t r) -> p t r", r=R).transpose([0, 2, 1])
            inv_b = (
                inv_t[b][:, ts].unsqueeze(1).broadcast_to([128, R, TPG])
            )
            nc.vector.tensor_mul(scores[:, :, ts], scps_v, inv_b)

        inv_tot_t = [None] * BLOC

        def softmax_tail_a(b):
            """exp + sum + 1/total (ACT + DVE + PE)."""
            scores = scores_t[b]          # [128, R, T] f32, pre-exp
            nc.scalar.activation(scores[:], scores[:], AF.Exp)
            s1 = smalls.tile([128, R], F32, tag="s1")
            nc.vector.reduce_sum(s1[:], scores[:], axis=mybir.AxisListType.X)
            s1_h = smalls.tile([128, R], F16)
            # pre-scale by 2^-15 so f16 output (x OUT_SCALE) stays normal-range
            nc.scalar.activation(s1_h[:], s1[:], AF.Copy, scale=1.0 / OUT_SCALE)
            tot_ps = rtps_pool.tile([128, R], F32, tag="prep")
            nc.tensor.matmul(tot_ps[:], ones_h[:], s1_h[:], start=True, stop=True)
            inv_tot = smalls.tile([128, R], F32, tag="invtot")
            nc.vector.reciprocal_approx_fast(inv_tot[:], tot_ps[:])
            inv_tot_t[b] = inv_tot

        def softmax_tail_b(b):
            """normalize quarters + output DMA on 4 queues (overlap the tail)."""
            scores = scores_t[b]
            inv_tot = inv_tot_t[b]
            scout = scout_pool.tile([128, R, T], F16, tag="scout")
            H = R // 4
            out_eng = [nc.sync, nc.scalar, nc.gpsimd, nc.sync]
            for h in range(4):
                hs = slice(h * H, (h + 1) * H)
                inv_b = inv_tot[:, hs].unsqueeze(2).broadcast_to([128, H, T])
                nc.vector.tensor_mul(scout[:, hs, :], scores[:, hs, :], inv_b)
                out_eng[h].dma_start(out[b, :, hs, :], scout[:, hs, :])

        def emit_group_mm(step):
            """transposes + memT drains + sim matmuls for one group."""
            b, g = divmod(step, NG)
            mem_g = mem_tiles[step]  # popped later by emit_norm (deferred DVE sq)
            scps = scps_pool.tile([128, TPG * R], F32)
            for q in range(NCH):  # 8-tile chunks (1024 n)
                mt_ps = mtps_pool.tile([128, CH * 128], F16)
                for j in range(CH):
                    tt = q * CH + j
                    nc.tensor.transpose(
                        mt_ps[:, j * 128 : (j + 1) * 128],
                        mem_g[:, tt, :],
                        id_h[:],
                    )
                mt_sb = mt_pool.tile([128, CH * 128], F16)
                de = MEMT_DRAIN[state["drain_i"] % len(MEMT_DRAIN)]
                state["drain_i"] += 1
                if de == "s":
                    nc.scalar.copy(mt_sb[:], mt_ps[:])
                else:
                    nc.vector.tensor_copy(mt_sb[:], mt_ps[:])

                for j in range(CH):
                    tt = q * CH + j
                    nc.tensor.matmul(
                        scps[:, tt * R : (tt + 1) * R],
                        mt_sb[:, j * 128 : (j + 1) * 128],
                        rvp_t[b][:],
                        start=True,
                        stop=True,
                    )
            scps_tiles[step] = scps

        # prologue: fill DMA pipeline, prep batch 0
        for s0 in range(DMA_AHEAD):
            issue_dma(s0)
        rv_prep(0)

        for step in range(NSTEP):
            b, g = divmod(step, NG)
            if g == 0:
                scores_new = score_pool.tile([128, R, T], F32, tag="scores")
                ss_new = ss_pool.tile([128, T], F32, tag="ss")
                inv_new = inv_pool.tile([128, T], F32, tag="inv")
                scores_t[b] = scores_new
                ss_t[b] = ss_new
                inv_t[b] = inv_new
            if step + DMA_AHEAD < NSTEP:
                issue_dma(step + DMA_AHEAD)
            # batch b+1 rv-prep midway through batch b
            if g == NG - 3 and b + 1 < BLOC:
                rv_prep(b + 1)

            emit_squares(step)
            # deferred norm chain (1 step) keeps DVE off the DMA chase
            if step >= 1:
                emit_norm(step - 1)
            # inv_nrm for the previous two groups, every other step
            if step >= 2 and step % 2 == 0:
                emit_inv(step - 2, step)
            # fused score drain (ready work) BEFORE this group's drains
            if step >= 2:
                emit_score_drain(step - 2)
            # batch b-1 softmax tail split across groups 2 and 3
            if g == 2 and b > 0:
                softmax_tail_a(b - 1)

            emit_group_mm(step)

            if g == 3 and b > 0:
                softmax_tail_b(b - 1)

        # epilogue: flush remaining norms, invs, drains + last batch softmax
        emit_norm(NSTEP - 1)
        emit_inv(NSTEP - 2, NSTEP)
        emit_score_drain(NSTEP - 2)
        emit_score_drain(NSTEP - 1)
        softmax_tail_a(BLOC - 1)
        softmax_tail_b(BLOC - 1)

    nc.compile()
    return nc


_program = None
last_results = None


def _get_program():
    global _program
    if _program is None:
        _program = build_program()
    return _program


def kernel(memory, read_strengths, read_vectors):
    memory = np.asarray(memory, dtype=np.float32)
    read_strengths = np.asarray(read_strengths, dtype=np.float32)
    read_vectors = np.asarray(read_vectors, dtype=np.float32)

    nc = _get_program()
    identity = np.eye(128, dtype=np.float32)
    ones_m = np.ones((128, 128), dtype=np.float32)
    in_maps = []
    for c in range(NCORES):
        sl = slice(c * BLOC, (c + 1) * BLOC)
        in_maps.append(
            {
                "memory": np.ascontiguousarray(memory[sl]),
                "read_vectors": np.ascontiguousarray(read_vectors[sl]),
                "read_strengths": np.ascontiguousarray(read_strengths[sl]),
                "identity": identity,
                "ones": ones_m,
            }
        )

    global last_results
    last_results = run_bass_kernel_spmd(nc, in_maps, list(range(NCORES)))
    res = last_results.results
    outs = []
    for c in range(NCORES):
        # (BLOC, 128, R, T) f16 * 2^15; tau = g*TPG + t; n = g*4096 + p*32 + t
        o = np.asarray(res[c]["out"]).astype(np.float32) / OUT_SCALE
        o = o.reshape(BLOC, 128, R, NG, TPG).transpose(0, 3, 1, 4, 2)
        outs.append(o.reshape(BLOC, N, R))
    return np.concatenate(outs, axis=0)
